# revision 9
# baseline (speedup 1.0000x reference)
import numpy as np
import ml_dtypes

# GCN 3-layer Trainium2 kernel — 8 cores, single launch, scatter-add design.
#
# norm factorization: norm = dinv[src]*dinv[dst], so the activation table is
# pre-scaled by dinv (t[i] = dinv[i]*h[i]) and aggregates are post-scaled by
# dinv[dst] after the dense transform ((D*A)@W = D*(A@W)). No per-edge norm.
#
# Per layer, per core (dst rows sharded 8 ways, 12500/core padded to 12544):
#   * dma_gather (gpsimd SWDGE) fetches per-edge source rows from the
#     replicated f16 table in DRAM (int16 idx -> 4 source ranges of 25088).
#   * dma_scatter_add accumulates rows into an f16 DRAM buffer by local dst
#     row. The DMA's RMW drops duplicate-row updates within one instruction,
#     so edges are bucketed by rank-within-(core,range,dst): every scatter
#     instruction touches each dst row at most once; buckets serialize via
#     tile WAW deps. Pad slots scatter to a dump row.
#   * Per 128-row window: PE-transpose the aggregate, dense matmul with W,
#     then dinv-scale + bias (+relu, + dinv pre-scale for the next table).
#   * AllGather replicates the next table across cores.
# Slot counts are padded to the max over the 8 cores per (range, rank-bucket)
# so one SPMD program serves every core (only the data differs).

N = 100000
F = 128
NC = 8
NPC = N // NC               # 12500
WIN = 128
NW = (NPC + WIN - 1) // WIN  # 98
NPAD = NW * WIN             # 12544
NFULL = NC * NPAD           # 100352
NR = 4
RANGE = NFULL // NR         # 25088
MAXG = 8192                 # max idx per SWDGE gather instruction
MAXS = 4096                 # max idx per scatter (RMW needs 2x descs)
DUMP = NPAD                 # scatter dump row for pad slots
NAGG = NPAD + 128           # agg rows incl. dump block (99*128)
KB = 64                     # rank-bucket cap

_cache = {}
EXEC_NS = []


def _radix_argsort(lo16, hi16):
    """Stable argsort by (hi16, lo16) via two uint16 radix passes."""
    p1 = np.argsort(lo16, kind="stable")
    p2 = np.argsort(hi16[p1], kind="stable")
    return p1[p2]


def _preprocess_phase1(edge_index):
    """Sorts + bucket sizes -> everything the program build needs."""
    ei = np.asarray(edge_index)
    src = ei[0].astype(np.int32, copy=False)
    dst = ei[1].astype(np.int32, copy=False)
    loop = np.arange(N, dtype=np.int32)
    src = np.concatenate([src, loop])
    dst = np.concatenate([dst, loop])
    E = src.shape[0]

    deg = np.bincount(dst, minlength=N).astype(np.float32)
    dinv = 1.0 / np.sqrt(deg)   # every node has a self loop -> deg >= 1

    srcp = (src // NPC) * NPAD + (src % NPC)
    r = (srcp // RANGE).astype(np.int32)
    srcl = (srcp % RANGE).astype(np.int16)
    core = dst // NPC
    dl = (dst - core * NPC).astype(np.int16)

    cr = core * NR + r                      # 0..31
    dlo = (dst & 0xFFFF).astype(np.uint16)
    dhi = (dst >> 16).astype(np.int32)      # 0 or 1
    # rank of each edge within its (core, range, dst) group
    o1 = _radix_argsort(dlo, (cr * 2 + dhi).astype(np.uint16))
    k1s = (cr * 131072 + dst)[o1]
    change = np.r_[True, k1s[1:] != k1s[:-1]]
    starts = np.flatnonzero(change)
    sizes = np.diff(np.r_[starts, E])
    rank_s = np.arange(E, dtype=np.int32) - np.repeat(starts, sizes)
    kk = np.empty(E, np.int32)
    kk[o1] = rank_s
    assert kk.max() < KB

    # order by (core, range, rank-bucket, dst)
    crk = cr * KB + kk                      # [0, NC*NR*KB)
    o2 = _radix_argsort(dlo, (crk * 2 + dhi).astype(np.uint16))

    cnt = np.bincount(crk, minlength=NC * NR * KB).reshape(NC, NR * KB)
    mx = cnt.max(axis=0)                    # [NR*KB]
    BS = ((mx + 127) // 128) * 128
    off2 = np.concatenate([[0], np.cumsum(BS)[:-1]]).astype(np.int64)
    NSLOT = int(BS.sum())

    plan = []
    for rr in range(NR):
        for k in range(KB):
            b = int(BS[rr * KB + k])
            if b == 0:
                continue
            base = int(off2[rr * KB + k])
            for c0 in range(0, b, MAXG):
                plan.append((rr, base + c0, min(MAXG, b - c0)))

    meta = dict(NSLOT=NSLOT, plan=plan)
    state = dict(E=E, core=core, srcl=srcl, dl=dl, crk=crk, o2=o2,
                 cnt=cnt, off2=off2, NSLOT=NSLOT, dinv=dinv)
    return meta, state


def _preprocess_phase2(st):
    """Slot array fills (run concurrently with the program build)."""
    E, NSLOT = st["E"], st["NSLOT"]
    o2, crk, cnt, off2 = st["o2"], st["crk"], st["cnt"], st["off2"]
    core_s = st["core"][o2]
    srcl_s = st["srcl"][o2]
    dl_s = st["dl"][o2]

    # rank within each (core, range, bucket) group in o2 order
    cntf = cnt.reshape(-1)                  # (core,(r,k)) C-order == o2 order
    startsf = np.cumsum(cntf) - cntf
    rank3 = np.arange(E, dtype=np.int64) - np.repeat(startsf, cntf)
    rk_s = crk[o2] % (NR * KB)
    slot = off2[rk_s] + rank3

    gidx = np.zeros((NC, NSLOT), np.int16)
    sidx = np.full((NC, NSLOT), DUMP, np.int16)
    gidx[core_s, slot] = srcl_s
    sidx[core_s, slot] = dl_s
    g16 = np.ascontiguousarray(
        gidx.reshape(NC, NSLOT // 16, 16).transpose(0, 2, 1))
    s16 = np.ascontiguousarray(
        sidx.reshape(NC, NSLOT // 16, 16).transpose(0, 2, 1))

    dinv = st["dinv"]
    dinv_pad = np.zeros((NC, NPAD), np.float32)
    dinv_pad[:, :NPC] = dinv.reshape(NC, NPC)
    dinv_sb = np.ascontiguousarray(
        dinv_pad.reshape(NC, NW, WIN).transpose(0, 2, 1)).astype(np.float16)
    return g16, s16, dinv_sb


def _preprocess(edge_index):
    meta, st = _preprocess_phase1(edge_index)
    g16, s16, dinv_sb = _preprocess_phase2(st)
    return meta, g16, s16, dinv_sb, st["dinv"]


def _build_program(meta):
    import concourse.mybir as mybir
    from concourse import bacc
    from concourse.tile import TileContext

    NSLOT = meta["NSLOT"]
    plan = meta["plan"]
    IC = NSLOT // 16

    nc = bacc.Bacc(None, target_bir_lowering=False, num_devices=NC)
    f16 = mybir.dt.float16
    i16 = mybir.dt.int16
    f32 = mybir.dt.float32

    xsh_d = nc.dram_tensor("xsh", [NPAD, F], f16, kind="ExternalInput")
    gid_d = nc.dram_tensor("gid", [16, IC], i16, kind="ExternalInput")
    sid_d = nc.dram_tensor("sid", [16, IC], i16, kind="ExternalInput")
    dinv_d = nc.dram_tensor("dinv", [128, NW], f16, kind="ExternalInput")
    W_d = nc.dram_tensor("W", [128, 3 * F], f16, kind="ExternalInput")
    brow_d = nc.dram_tensor("brow", [1, 3 * F], f32, kind="ExternalInput")
    out_d = nc.dram_tensor("out", [NPAD, F], f16, kind="ExternalOutput")
    act_a = nc.dram_tensor("act_a", [NFULL, F], f16)
    act_b = nc.dram_tensor("act_b", [NFULL, F], f16)
    agg_d = nc.dram_tensor("agg", [NAGG, F], f16)
    zz_d = nc.dram_tensor("zz", [NAGG, F], f16)
    shard = nc.dram_tensor("shard", [NPAD, F], f16)

    rg = [list(range(NC))]

    with TileContext(nc) as tc:
        with (
            tc.tile_pool(name="res", bufs=1) as res,
            tc.tile_pool(name="gb", bufs=3) as gb,
            tc.tile_pool(name="wp", bufs=3) as wp,
            tc.tile_pool(name="psp", bufs=2, space="PSUM") as psp,
        ):
            gid_s = res.tile([128, IC], i16)
            sid_s = res.tile([128, IC], i16)
            W_s = res.tile([128, 3 * F], f16)
            dinv_s = res.tile([128, NW], f16)
            brow_s = res.tile([1, 3 * F], f32)
            for k in range(8):
                nc.sync.dma_start(out=gid_s[16 * k:16 * (k + 1), :],
                                  in_=gid_d[:, :])
                nc.sync.dma_start(out=sid_s[16 * k:16 * (k + 1), :],
                                  in_=sid_d[:, :])
            nc.sync.dma_start(out=W_s[:, :], in_=W_d[:, :])
            nc.sync.dma_start(out=dinv_s[:, :], in_=dinv_d[:, :])
            nc.sync.dma_start(out=brow_s[:, :], in_=brow_d[:, :])

            # bias broadcast [128, 3F] via ones outer product
            ones_s = res.tile([1, 128], f16)
            nc.vector.memset(ones_s[:, :], 1.0)
            brow_h = res.tile([1, 3 * F], f16)
            nc.vector.tensor_copy(out=brow_h[:, :], in_=brow_s[:, :])
            psB = psp.tile([128, 3 * F], f32, tag="psB")
            nc.tensor.matmul(psB[:, :], ones_s[:, :], brow_h[:, :],
                             start=True, stop=True)
            biasB = res.tile([128, 3 * F], f32)
            nc.vector.tensor_copy(out=biasB[:, :], in_=psB[:, :])

            # identity for PE transpose
            ic_t = res.tile([128, 128], f16)
            ir_t = res.tile([128, 128], f16)
            nc.gpsimd.iota(ic_t[:, :], pattern=[[1, 128]], base=0,
                           channel_multiplier=0,
                           allow_small_or_imprecise_dtypes=True)
            nc.gpsimd.iota(ir_t[:, :], pattern=[[0, 128]], base=0,
                           channel_multiplier=1,
                           allow_small_or_imprecise_dtypes=True)
            ident = res.tile([128, 128], f16)
            nc.vector.tensor_tensor(out=ident[:, :], in0=ic_t[:, :],
                                    in1=ir_t[:, :],
                                    op=mybir.AluOpType.is_equal)

            # zeros source for agg reset
            zero_s = res.tile([128, F], f16)
            nc.vector.memset(zero_s[:, :], 0.0)
            for w in range(NAGG // 128):
                nc.sync.dma_start(out=zz_d[w * 128:(w + 1) * 128, :],
                                  in_=zero_s[:, :])

            nc.sync.dma_start(out=shard[:, :], in_=xsh_d[:, :])
            nc.gpsimd.collective_compute(
                "AllGather", mybir.AluOpType.bypass, replica_groups=rg,
                ins=[shard.ap().opt()], outs=[act_a.ap().opt()],
            )

            for l in range(3):
                tab = act_a if l % 2 == 0 else act_b
                nc.sync.dma_start(out=agg_d[:, :], in_=zz_d[:, :])
                for (rr, s0, n) in plan:
                    cn = n // 128
                    g = gb.tile([128, MAXG // 128, F], f16, tag="g")
                    nc.gpsimd.dma_gather(
                        out_ap=g[:, :cn, :],
                        in_ap=tab[rr * RANGE:(rr + 1) * RANGE, :],
                        idxs_ap=gid_s[:, s0 // 16:(s0 + n) // 16],
                        num_idxs=n,
                        num_idxs_reg=n,
                        elem_size=F,
                        single_packet=False,
                    )
                    for c0 in range(0, n, MAXS):
                        m = min(MAXS, n - c0)
                        nc.gpsimd.dma_scatter_add(
                            agg_d[:, :],
                            g[:, c0 // 128:(c0 + m) // 128, :],
                            sid_s[:, (s0 + c0) // 16:(s0 + c0 + m) // 16],
                            m,
                            m,
                            F,
                        )
                for w in range(NW):
                    a_t = wp.tile([128, F], f16, tag="a")
                    nc.sync.dma_start(out=a_t[:, :],
                                      in_=agg_d[w * 128:(w + 1) * 128, :])
                    tr = psp.tile([128, F], f16, tag="tr")
                    nc.tensor.transpose(tr[:, :], a_t[:, :], ident[:, :])
                    zT = wp.tile([128, F], f16, tag="zT")
                    nc.vector.tensor_copy(out=zT[:, :], in_=tr[:, :])
                    p2 = psp.tile([128, F], f32, tag="p2")
                    nc.tensor.matmul(p2[:, :], zT[:, :],
                                     W_s[:, l * F:(l + 1) * F],
                                     start=True, stop=True)
                    dvc = dinv_s[:, w:w + 1]
                    dvb = dvc.to_broadcast([128, 1, F])
                    if l < 2:
                        e1 = wp.tile([128, F], f32, tag="e1")
                        nc.vector.scalar_tensor_tensor(
                            out=e1[:, :], in0=p2[:, :], scalar=dvc,
                            in1=biasB[:, l * F:(l + 1) * F],
                            op0=mybir.AluOpType.mult,
                            op1=mybir.AluOpType.add)
                        o_t = wp.tile([128, F], f16, tag="o")
                        nc.vector.scalar_tensor_tensor(
                            out=o_t[:, :], in0=e1[:, :], scalar=0.0,
                            in1=dvb,
                            op0=mybir.AluOpType.max,
                            op1=mybir.AluOpType.mult)
                        nc.sync.dma_start(
                            out=shard[w * WIN:(w + 1) * WIN, :],
                            in_=o_t[:, :])
                    else:
                        o_t = wp.tile([128, F], f16, tag="o")
                        nc.vector.scalar_tensor_tensor(
                            out=o_t[:, :], in0=p2[:, :], scalar=dvc,
                            in1=biasB[:, l * F:(l + 1) * F],
                            op0=mybir.AluOpType.mult,
                            op1=mybir.AluOpType.add)
                        nc.sync.dma_start(
                            out=out_d[w * WIN:(w + 1) * WIN, :],
                            in_=o_t[:, :])
                if l < 2:
                    dst_t = act_b if l % 2 == 0 else act_a
                    nc.gpsimd.collective_compute(
                        "AllGather", mybir.AluOpType.bypass, replica_groups=rg,
                        ins=[shard.ap().opt()], outs=[dst_t.ap().opt()],
                    )
    nc.compile()
    return nc


def kernel(x, edge_index, W1, b1, W2, b2, W3, b3):
    import threading
    from concourse.bass_utils import run_bass_kernel_spmd

    f16 = np.float16
    if "prep" in _cache:
        meta, g16, s16, dinv_sb, dinv, prog = _cache["prep"]
        xs = np.asarray(x, np.float32) * dinv[:, None]
        xpad = np.zeros((NC, NPAD, F), dtype=f16)
        xpad[:, :NPC, :] = xs.reshape(NC, NPC, F).astype(f16)
    else:
        meta, st = _preprocess_phase1(edge_index)
        box = {}

        def work():
            box["fills"] = _preprocess_phase2(st)
            xs = np.asarray(x, np.float32) * st["dinv"][:, None]
            xpad = np.zeros((NC, NPAD, F), dtype=f16)
            xpad[:, :NPC, :] = xs.reshape(NC, NPC, F).astype(f16)
            box["xpad"] = xpad

        th = threading.Thread(target=work)
        th.start()
        prog = _build_program(meta)
        th.join()
        g16, s16, dinv_sb = box["fills"]
        xpad = box["xpad"]
        dinv = st["dinv"]
        _cache["prep"] = (meta, g16, s16, dinv_sb, dinv, prog)

    Wtile = np.concatenate(
        [np.asarray(Wl, np.float32).astype(f16) for Wl in (W1, W2, W3)],
        axis=1)
    brow = np.concatenate(
        [np.asarray(bl, np.float32) for bl in (b1, b2, b3)])[None, :]

    in_maps = []
    for c in range(NC):
        in_maps.append({
            "xsh": np.ascontiguousarray(xpad[c]),
            "gid": np.ascontiguousarray(g16[c]),
            "sid": np.ascontiguousarray(s16[c]),
            "dinv": np.ascontiguousarray(dinv_sb[c]),
            "W": np.ascontiguousarray(Wtile),
            "brow": np.ascontiguousarray(brow.astype(np.float32)),
        })
    import time
    t0 = time.perf_counter_ns()
    res = run_bass_kernel_spmd(prog, in_maps, list(range(NC)))
    t1 = time.perf_counter_ns()
    EXEC_NS.append(res.exec_time_ns if getattr(res, "exec_time_ns", None)
                   else t1 - t0)
    outs = []
    for c in range(NC):
        r = res.results[c]
        if isinstance(r, dict):
            r = r["out"]
        elif isinstance(r, (list, tuple)):
            r = r[0]
        outs.append(np.asarray(r)[:NPC])
    return np.concatenate(outs, axis=0).astype(np.float32)


# revision 14
# speedup vs baseline: 1.0464x; 1.0464x over previous
import numpy as np
import ml_dtypes

# GCN 3-layer Trainium2 kernel — 8 cores, single launch, scatter-add design.
#
# norm factorization: norm = dinv[src]*dinv[dst], so the activation table is
# pre-scaled by dinv (t[i] = dinv[i]*h[i]) and aggregates are post-scaled by
# dinv[dst] after the dense transform ((D*A)@W = D*(A@W)). No per-edge norm.
#
# Per layer, per core (dst rows sharded 8 ways, 12500/core padded to 12544):
#   * dma_gather (gpsimd SWDGE) fetches per-edge source rows from the
#     replicated f16 table in DRAM (int16 idx -> 4 source ranges of 25088).
#   * dma_scatter_add accumulates rows into an f16 DRAM buffer by local dst
#     row. The DMA's RMW drops duplicate-row updates within one instruction,
#     so edges are bucketed by rank-within-(core,range,dst): every scatter
#     instruction touches each dst row at most once; buckets serialize via
#     tile WAW deps. Pad slots scatter to a dump row.
#   * Per 128-row window: PE-transpose the aggregate, dense matmul with W,
#     then dinv-scale + bias (+relu, + dinv pre-scale for the next table).
#   * AllGather replicates the next table across cores.
# Slot counts are padded to the max over the 8 cores per (range, rank-bucket)
# so one SPMD program serves every core (only the data differs).

N = 100000
F = 128
NC = 8
NPC = N // NC               # 12500
WIN = 128
NW = (NPC + WIN - 1) // WIN  # 98
NPAD = NW * WIN             # 12544
NFULL = NC * NPAD           # 100352
NR = 4
RANGE = NFULL // NR         # 25088
MAXG = 8192                 # max idx per SWDGE gather instruction
MAXS = 4096                 # max idx per scatter (RMW needs 2x descs)
DUMP = NPAD                 # scatter dump row for pad slots
NAGG = NPAD + 128           # agg rows incl. dump block (99*128)
KB = 64                     # rank-bucket cap

_cache = {}
EXEC_NS = []


def _radix_argsort(lo16, hi16):
    """Stable argsort by (hi16, lo16) via two uint16 radix passes."""
    p1 = np.argsort(lo16, kind="stable")
    p2 = np.argsort(hi16[p1], kind="stable")
    return p1[p2]


def _preprocess_phase1(edge_index):
    """Sorts + bucket sizes -> everything the program build needs."""
    ei = np.asarray(edge_index)
    src = ei[0].astype(np.int32, copy=False)
    dst = ei[1].astype(np.int32, copy=False)
    loop = np.arange(N, dtype=np.int32)
    src = np.concatenate([src, loop])
    dst = np.concatenate([dst, loop])
    E = src.shape[0]

    deg = np.bincount(dst, minlength=N).astype(np.float32)
    dinv = 1.0 / np.sqrt(deg)   # every node has a self loop -> deg >= 1

    srcp = (src // NPC) * NPAD + (src % NPC)
    r = (srcp // RANGE).astype(np.int32)
    srcl = (srcp % RANGE).astype(np.int16)
    core = dst // NPC
    dl = (dst - core * NPC).astype(np.int16)

    cr = core * NR + r                      # 0..31
    dlo = (dst & 0xFFFF).astype(np.uint16)
    dhi = (dst >> 16).astype(np.int32)      # 0 or 1
    # rank of each edge within its (core, range, dst) group
    o1 = _radix_argsort(dlo, (cr * 2 + dhi).astype(np.uint16))
    k1s = (cr * 131072 + dst)[o1]
    change = np.r_[True, k1s[1:] != k1s[:-1]]
    starts = np.flatnonzero(change)
    sizes = np.diff(np.r_[starts, E])
    rank_s = np.arange(E, dtype=np.int32) - np.repeat(starts, sizes)
    kk = np.empty(E, np.int32)
    kk[o1] = rank_s
    assert kk.max() < KB

    # order by (core, range, rank-bucket, dst)
    crk = cr * KB + kk                      # [0, NC*NR*KB)
    o2 = _radix_argsort(dlo, (crk * 2 + dhi).astype(np.uint16))

    cnt = np.bincount(crk, minlength=NC * NR * KB).reshape(NC, NR * KB)
    mx = cnt.max(axis=0)                    # [NR*KB]
    BS = ((mx + 127) // 128) * 128
    off2 = np.concatenate([[0], np.cumsum(BS)[:-1]]).astype(np.int64)
    NSLOT = int(BS.sum())

    plan = []
    for rr in range(NR):
        for k in range(KB):
            b = int(BS[rr * KB + k])
            if b == 0:
                continue
            base = int(off2[rr * KB + k])
            for c0 in range(0, b, MAXG):
                plan.append((rr, base + c0, min(MAXG, b - c0)))

    meta = dict(NSLOT=NSLOT, plan=plan)
    state = dict(E=E, core=core, srcl=srcl, dl=dl, crk=crk, o2=o2,
                 cnt=cnt, off2=off2, NSLOT=NSLOT, dinv=dinv)
    return meta, state


def _preprocess_phase2(st):
    """Slot array fills (run concurrently with the program build)."""
    E, NSLOT = st["E"], st["NSLOT"]
    o2, crk, cnt, off2 = st["o2"], st["crk"], st["cnt"], st["off2"]
    core_s = st["core"][o2]
    srcl_s = st["srcl"][o2]
    dl_s = st["dl"][o2]

    # rank within each (core, range, bucket) group in o2 order
    cntf = cnt.reshape(-1)                  # (core,(r,k)) C-order == o2 order
    startsf = np.cumsum(cntf) - cntf
    rank3 = np.arange(E, dtype=np.int64) - np.repeat(startsf, cntf)
    rk_s = crk[o2] % (NR * KB)
    slot = off2[rk_s] + rank3

    gidx = np.zeros((NC, NSLOT), np.int16)
    sidx = np.full((NC, NSLOT), DUMP, np.int16)
    gidx[core_s, slot] = srcl_s
    sidx[core_s, slot] = dl_s
    g16 = np.ascontiguousarray(
        gidx.reshape(NC, NSLOT // 16, 16).transpose(0, 2, 1))
    s16 = np.ascontiguousarray(
        sidx.reshape(NC, NSLOT // 16, 16).transpose(0, 2, 1))

    dinv = st["dinv"]
    dinv_pad = np.zeros((NC, NPAD), np.float32)
    dinv_pad[:, :NPC] = dinv.reshape(NC, NPC)
    dinv_sb = np.ascontiguousarray(
        dinv_pad.reshape(NC, NW, WIN).transpose(0, 2, 1)).astype(np.float16)
    return g16, s16, dinv_sb


def _preprocess(edge_index):
    meta, st = _preprocess_phase1(edge_index)
    g16, s16, dinv_sb = _preprocess_phase2(st)
    return meta, g16, s16, dinv_sb, st["dinv"]


def _build_program(meta):
    import concourse.mybir as mybir
    from concourse import bacc
    from concourse.tile import TileContext

    NSLOT = meta["NSLOT"]
    plan = meta["plan"]
    IC = NSLOT // 16

    nc = bacc.Bacc(None, target_bir_lowering=False, num_devices=NC)
    f16 = mybir.dt.float16
    i16 = mybir.dt.int16
    f32 = mybir.dt.float32

    xsh_d = nc.dram_tensor("xsh", [NPAD, F], f16, kind="ExternalInput")
    gid_d = nc.dram_tensor("gid", [16, IC], i16, kind="ExternalInput")
    sid_d = nc.dram_tensor("sid", [16, IC], i16, kind="ExternalInput")
    wi_d = nc.dram_tensor("wi", [16, 8], i16, kind="ExternalInput")
    dinv_d = nc.dram_tensor("dinv", [128, NW], f16, kind="ExternalInput")
    W_d = nc.dram_tensor("W", [128, 3 * F], f16, kind="ExternalInput")
    brow_d = nc.dram_tensor("brow", [1, 3 * F], f32, kind="ExternalInput")
    out_d = nc.dram_tensor("out", [NPAD, F], f16, kind="ExternalOutput")
    act_a = nc.dram_tensor("act_a", [NFULL, F], f16)
    act_b = nc.dram_tensor("act_b", [NFULL, F], f16)
    agg_d = nc.dram_tensor("agg", [NAGG, F], f16)
    zz_d = nc.dram_tensor("zz", [NAGG, F], f16)
    shard = nc.dram_tensor("shard", [NPAD, F], f16)

    rg = [list(range(NC))]

    with TileContext(nc) as tc:
        with (
            tc.tile_pool(name="res", bufs=1) as res,
            tc.tile_pool(name="gb", bufs=3) as gb,
            tc.tile_pool(name="wp", bufs=3) as wp,
            tc.tile_pool(name="psp", bufs=2, space="PSUM") as psp,
        ):
            gid_s = res.tile([128, IC], i16)
            sid_s = res.tile([128, IC], i16)
            W_s = res.tile([128, 3 * F], f16)
            dinv_s = res.tile([128, NW], f16)
            brow_s = res.tile([1, 3 * F], f32)
            wi_s = res.tile([128, 8], i16)
            for k in range(8):
                nc.sync.dma_start(out=gid_s[16 * k:16 * (k + 1), :],
                                  in_=gid_d[:, :])
                nc.sync.dma_start(out=sid_s[16 * k:16 * (k + 1), :],
                                  in_=sid_d[:, :])
                nc.sync.dma_start(out=wi_s[16 * k:16 * (k + 1), :],
                                  in_=wi_d[:, :])
            nc.sync.dma_start(out=W_s[:, :], in_=W_d[:, :])
            nc.sync.dma_start(out=dinv_s[:, :], in_=dinv_d[:, :])
            nc.sync.dma_start(out=brow_s[:, :], in_=brow_d[:, :])

            # bias broadcast [128, 3F] via ones outer product
            ones_s = res.tile([1, 128], f16)
            nc.vector.memset(ones_s[:, :], 1.0)
            brow_h = res.tile([1, 3 * F], f16)
            nc.vector.tensor_copy(out=brow_h[:, :], in_=brow_s[:, :])
            psB = psp.tile([128, 3 * F], f32, tag="psB")
            nc.tensor.matmul(psB[:, :], ones_s[:, :], brow_h[:, :],
                             start=True, stop=True)
            biasB = res.tile([128, 3 * F], f32)
            nc.vector.tensor_copy(out=biasB[:, :], in_=psB[:, :])

            # zeros source for agg reset
            zero_s = res.tile([128, F], f16)
            nc.vector.memset(zero_s[:, :], 0.0)
            for w in range(NAGG // 128):
                nc.sync.dma_start(out=zz_d[w * 128:(w + 1) * 128, :],
                                  in_=zero_s[:, :])

            nc.sync.dma_start(out=shard[:, :], in_=xsh_d[:, :])
            nc.gpsimd.collective_compute(
                "AllGather", mybir.AluOpType.bypass, replica_groups=rg,
                ins=[shard.ap().opt()], outs=[act_a.ap().opt()],
            )

            for l in range(3):
                tab = act_a if l % 2 == 0 else act_b
                nc.sync.dma_start(out=agg_d[:, :], in_=zz_d[:, :])
                for (rr, s0, n) in plan:
                    cn = n // 128
                    g = gb.tile([128, MAXG // 128, F], f16, tag="g")
                    nc.gpsimd.dma_gather(
                        out_ap=g[:, :cn, :],
                        in_ap=tab[rr * RANGE:(rr + 1) * RANGE, :],
                        idxs_ap=gid_s[:, s0 // 16:(s0 + n) // 16],
                        num_idxs=n,
                        num_idxs_reg=n,
                        elem_size=F,
                        single_packet=False,
                    )
                    for c0 in range(0, n, MAXS):
                        m = min(MAXS, n - c0)
                        nc.gpsimd.dma_scatter_add(
                            agg_d[:, :],
                            g[:, c0 // 128:(c0 + m) // 128, :],
                            sid_s[:, (s0 + c0) // 16:(s0 + c0 + m) // 16],
                            m,
                            m,
                            F,
                        )
                for w in range(NW):
                    zT = wp.tile([128, F], f16, tag="zT")
                    nc.gpsimd.dma_gather(
                        out_ap=zT[:, :].rearrange("p (c f) -> p c f", f=F),
                        in_ap=agg_d[w * 128:(w + 1) * 128, :],
                        idxs_ap=wi_s[:, :],
                        num_idxs=128,
                        num_idxs_reg=128,
                        elem_size=F,
                        transpose=True,
                        single_packet=False,
                    )
                    p2 = psp.tile([128, F], f32, tag="p2")
                    nc.tensor.matmul(p2[:, :], zT[:, :],
                                     W_s[:, l * F:(l + 1) * F],
                                     start=True, stop=True)
                    dvc = dinv_s[:, w:w + 1]
                    dvb = dvc.to_broadcast([128, 1, F])
                    if l < 2:
                        e1 = wp.tile([128, F], f32, tag="e1")
                        nc.vector.scalar_tensor_tensor(
                            out=e1[:, :], in0=p2[:, :], scalar=dvc,
                            in1=biasB[:, l * F:(l + 1) * F],
                            op0=mybir.AluOpType.mult,
                            op1=mybir.AluOpType.add)
                        o_t = wp.tile([128, F], f16, tag="o")
                        nc.vector.scalar_tensor_tensor(
                            out=o_t[:, :], in0=e1[:, :], scalar=0.0,
                            in1=dvb,
                            op0=mybir.AluOpType.max,
                            op1=mybir.AluOpType.mult)
                        nc.sync.dma_start(
                            out=shard[w * WIN:(w + 1) * WIN, :],
                            in_=o_t[:, :])
                    else:
                        o_t = wp.tile([128, F], f16, tag="o")
                        nc.vector.scalar_tensor_tensor(
                            out=o_t[:, :], in0=p2[:, :], scalar=dvc,
                            in1=biasB[:, l * F:(l + 1) * F],
                            op0=mybir.AluOpType.mult,
                            op1=mybir.AluOpType.add)
                        nc.sync.dma_start(
                            out=out_d[w * WIN:(w + 1) * WIN, :],
                            in_=o_t[:, :])
                if l < 2:
                    dst_t = act_b if l % 2 == 0 else act_a
                    nc.gpsimd.collective_compute(
                        "AllGather", mybir.AluOpType.bypass, replica_groups=rg,
                        ins=[shard.ap().opt()], outs=[dst_t.ap().opt()],
                    )
    nc.compile()
    return nc


def kernel(x, edge_index, W1, b1, W2, b2, W3, b3):
    import threading
    from concourse.bass_utils import run_bass_kernel_spmd

    f16 = np.float16
    if "prep" in _cache:
        meta, g16, s16, dinv_sb, dinv, prog = _cache["prep"]
        xs = np.asarray(x, np.float32) * dinv[:, None]
        xpad = np.zeros((NC, NPAD, F), dtype=f16)
        xpad[:, :NPC, :] = xs.reshape(NC, NPC, F).astype(f16)
    else:
        meta, st = _preprocess_phase1(edge_index)
        box = {}

        def work():
            box["fills"] = _preprocess_phase2(st)
            xs = np.asarray(x, np.float32) * st["dinv"][:, None]
            xpad = np.zeros((NC, NPAD, F), dtype=f16)
            xpad[:, :NPC, :] = xs.reshape(NC, NPC, F).astype(f16)
            box["xpad"] = xpad

        th = threading.Thread(target=work)
        th.start()
        prog = _build_program(meta)
        th.join()
        g16, s16, dinv_sb = box["fills"]
        xpad = box["xpad"]
        dinv = st["dinv"]
        _cache["prep"] = (meta, g16, s16, dinv_sb, dinv, prog)

    Wtile = np.concatenate(
        [np.asarray(Wl, np.float32).astype(f16) for Wl in (W1, W2, W3)],
        axis=1)
    brow = np.concatenate(
        [np.asarray(bl, np.float32) for bl in (b1, b2, b3)])[None, :]

    wi = np.ascontiguousarray(np.arange(128, dtype=np.int16).reshape(8, 16).T)
    in_maps = []
    for c in range(NC):
        in_maps.append({
            "xsh": np.ascontiguousarray(xpad[c]),
            "gid": np.ascontiguousarray(g16[c]),
            "sid": np.ascontiguousarray(s16[c]),
            "wi": wi,
            "dinv": np.ascontiguousarray(dinv_sb[c]),
            "W": np.ascontiguousarray(Wtile),
            "brow": np.ascontiguousarray(brow.astype(np.float32)),
        })
    import time
    t0 = time.perf_counter_ns()
    res = run_bass_kernel_spmd(prog, in_maps, list(range(NC)))
    t1 = time.perf_counter_ns()
    EXEC_NS.append(res.exec_time_ns if getattr(res, "exec_time_ns", None)
                   else t1 - t0)
    outs = []
    for c in range(NC):
        r = res.results[c]
        if isinstance(r, dict):
            r = r["out"]
        elif isinstance(r, (list, tuple)):
            r = r[0]
        outs.append(np.asarray(r)[:NPC])
    return np.concatenate(outs, axis=0).astype(np.float32)
